# revision 32
# baseline (speedup 1.0000x reference)
"""Trainium2 Bass kernel for nn_CriticUAVob (attention-pool critic).

Math per item b (4096 total): two attention-pool branches over s_b [N=128, 3]
followed by a small MLP.  With s' = [s, 1] (N x 4) and A = Wq' Wk'^T / 4:

    S = s' A s'^T,  U = exp(S),  Z[q] = sum_kk U[q,kk]
    pooled = (1/N) * t^T Wv',  t[k] = sum_q (sum_kk U[q,kk] s'[kk,k]) / Z[q]

Per-core layout (512 items = 128 quads = 32 groups of 4 quads):
  - group inputs qtss4 [128, 1152]: quad jj of the group occupies SBUF
    partitions 32jj..32jj+16 with [qt | sst] (block-diag A^T s'^T + stacked
    s'^T); st matmuls run row-tiled (tile_position (32jj, 0)), 2 x N=512 per
    quad -> ps_st [128, 1024] = S^T with cols (b, i, q).
  - exp: one ScalarE activation [128, 1024] per quad (identity layout) ->
    ut bf16.  This is the kernel's roofline: ~1.05us x 128 quads.
  - G matmuls: 4 per quad, lhsT = zero-padded s' variants [128, 32], output
    col-strip 32jj of a shared gbig [128, 256] PSUM tile (cols (b, q)),
    accumulating all 4 items; the s' ones-column makes row (i,3) = Z.
  - per-group chain on full-width [128, 256] tiles: bf16 cast (DVE), rep128
    matmul broadcasts each item's Z row over its 4 rows (PE), fast reciprocal
    (DVE), then two fused tensor_tensor_reduce (DVE) produce t columns of
    tbig [128, 64] directly.
  - MLP: first layer folds Wv'/N and W1 into C [4,128]; 32 row-tiled matmuls
    with zero-padded C variants read tbig in place; tanh/tanh/linear finish,
    and the item permutation (jj, i, g) is undone for free in the output DMA.
"""
import os
import sys
import numpy as np
import ml_dtypes

sys.path.insert(0, "/opt/trn_rl_repo")

import concourse.bass as bass
import concourse.tile as tile
from concourse import bacc, mybir
from concourse import bass_utils

N_CORES = 8
B = 4096
N = 128
BC = B // N_CORES          # 512 items per core
QUADS = BC // 4            # 128 quads of 4 items
NG = QUADS // 4            # 32 groups of 4 quads
F32 = mybir.dt.float32
BF16 = mybir.dt.bfloat16
AF = mybir.ActivationFunctionType
ALU = mybir.AluOpType
BF = ml_dtypes.bfloat16

_cache = {}


def _build():
    nc = bacc.Bacc(
        "TRN2",
        target_bir_lowering=False,
        debug=False,
        enable_asserts=False,
        num_devices=N_CORES,
    )
    qtss4_t = nc.dram_tensor("qtss4", [NG, 16, 4608], BF16, kind="ExternalInput")
    snatp4_t = nc.dram_tensor("snatp4", [NG, 128, 512], BF16, kind="ExternalInput")
    rep_t = nc.dram_tensor("rep128", [128, 128], BF16, kind="ExternalInput")
    cvar_t = nc.dram_tensor("cvar", [128, 4096], BF16, kind="ExternalInput")
    w2_t = nc.dram_tensor("w2", [128, 128], BF16, kind="ExternalInput")
    w3_t = nc.dram_tensor("w3", [128, 1], BF16, kind="ExternalInput")
    b1_t = nc.dram_tensor("b1", [128, 1], F32, kind="ExternalInput")
    b2_t = nc.dram_tensor("b2", [128, 1], F32, kind="ExternalInput")
    b3_t = nc.dram_tensor("b3rep", [1, BC], F32, kind="ExternalInput")
    out_t = nc.dram_tensor("out", [BC, 1], F32, kind="ExternalOutput")

    qtss4_ap = qtss4_t.ap()
    snatp4_ap = snatp4_t.ap()

    with tile.TileContext(nc) as tc:
        with (
            tc.tile_pool(name="singles", bufs=1) as singles,
            tc.tile_pool(name="qsb", bufs=2) as qsb,
            tc.tile_pool(name="pst", bufs=3, space="PSUM") as pst,
            tc.tile_pool(name="pg", bufs=2, space="PSUM") as pg,
        ):
            # singles DMAs are deferred into the loop on the gpsimd queue so
            # neither the scalar queue (exp stream) nor the first qtss/snatp
            # loads are delayed
            rep128 = singles.tile([128, 128], BF16)
            cvar = singles.tile([128, 4096], BF16)
            w2 = singles.tile([128, 128], BF16)
            w3 = singles.tile([128, 1], BF16)
            b1 = singles.tile([128, 1], F32)
            b2 = singles.tile([128, 1], F32)
            b3r = singles.tile([1, BC], F32)
            single_dmas = [
                (rep128, rep_t), (cvar, cvar_t), (w2, w2_t), (w3, w3_t),
                (b1, b1_t), (b2, b2_t), (b3r, b3_t),
            ]
            # t accumulator: rows (jj, i, k) in 32-strips, cols (g, b)
            tbig = singles.tile([128, 2 * NG], F32)

            qtssT, snatpT, ps_stT, utT, gbigT, gcastT, rbigT = (
                {}, {}, {}, {}, {}, {}, {},
            )

            def issue_dma(g):
                qtssT[g] = qsb.tile([16, 4608], BF16, tag="qtss", bufs=3,
                                    name="qtss")
                nc.sync.dma_start(qtssT[g][:], qtss4_ap[g])
                snatpT[g] = qsb.tile([128, 512], BF16, tag="snatp", bufs=4,
                                     name="snatp")
                nc.gpsimd.dma_start(snatpT[g][:], snatp4_ap[g])

            # Software pipeline over iteration PAIRS: per pair (j, j+1) the PE
            # runs [rep (if due)] [8 G matmuls] [4 st matmuls] — quad q's
            # G-item-i matmul lands at iteration q+2+i, so consecutive G
            # matmuls come from different quads = distinct PSUM col-strips and
            # stream concurrently in the PE array (array packing).  Pairing
            # halves the row/col-tiling mode-switch drains.
            for j2 in range(0, QUADS + 12, 2):
                if j2 < QUADS and j2 % 4 == 0:
                    g = j2 // 4
                    if g == 0:
                        issue_dma(0)
                    if g + 1 < NG:
                        issue_dma(g + 1)
                if j2 == 2:
                    for tl, dt_ in single_dmas:
                        nc.gpsimd.dma_start(tl[:], dt_.ap())

                # chain part 2 first (correct WAR order vs gbig slot reuse):
                # rep (PE) reads gcast from the previous pair, then recip +
                # mul/reduce (DVE).  zrep shares gbig's PSUM bank.
                for j in (j2, j2 + 1):
                    if j >= 10 and (j - 10) % 4 == 0 and (j - 10) // 4 < NG:
                        gr = (j - 10) // 4
                        nc.tensor.matmul(gbigT[gr][:, 256:512], rep128[:],
                                         gcastT[gr][:])
                        rbig = qsb.tile([128, 256], F32, tag="rbig",
                                        name="rbig")
                        rbigT[gr] = rbig
                        nc.vector.reciprocal_approx_fast(
                            rbig[:], gbigT[gr][:, 256:512])
                        pgm = qsb.tile([128, 256], F32, tag="scr", name="pgm")
                        nc.vector.tensor_mul(pgm[:], gbigT[gr][:, 0:256],
                                             rbig[:])
                        pg3 = pgm[:].rearrange("p (b q) -> p b q", b=2)
                        nc.vector.tensor_reduce(
                            tbig[:, 2 * gr:2 * (gr + 1)], pg3,
                            axis=mybir.AxisListType.X, op=ALU.add,
                        )
                        del gbigT[gr], gcastT[gr], rbigT[gr]

                # G: 8 matmuls (item i of quad j-3-i for both iterations)
                for j in (j2, j2 + 1):
                    for i in range(4):
                        qg = j - 3 - i
                        if not (0 <= qg < QUADS):
                            continue
                        gg, jj = qg // 4, qg % 4
                        if jj == 0 and i == 0:
                            gbigT[gg] = pg.tile([128, 512], F32, tag="gbig",
                                                name="gbig")
                        gbig = gbigT[gg]
                        ut_r = utT[qg][:].rearrange("p (b i q) -> p i b q",
                                                    b=2, i=4)
                        sn = snatpT[gg]
                        nc.tensor.matmul(
                            gbig[32 * jj:32 * (jj + 1), 0:256],
                            sn[:, 128 * jj + 32 * i:
                               128 * jj + 32 * (i + 1)],
                            ut_r[:, i],
                            start=(i == 0),
                            stop=(i == 3),
                            tile_position=(0, 32 * jj),
                        )
                        if i == 3:
                            del utT[qg]
                            if jj == 3:
                                # chain part 1: bf16 cast for rep's rhs
                                gcast = qsb.tile([128, 256], BF16,
                                                 tag="gcast", name="gcast")
                                gcastT[gg] = gcast
                                nc.vector.tensor_copy(gcast[:],
                                                      gbig[:, 0:256])

                # st(j2), st(j2+1): two matmuls each
                for j in (j2, j2 + 1):
                    if j < QUADS:
                        g, jj = j // 4, j % 4
                        qt = qtssT[g]
                        c0 = 1152 * jj
                        sst = qt[0:16, c0 + 1024:c0 + 1152]
                        ps_st = pst.tile([128, 1024], F32, tag="st",
                                         name="ps_st")
                        ps_stT[j] = ps_st
                        nc.tensor.matmul(
                            ps_st[:, 0:512], sst, qt[0:16, c0:c0 + 512])
                        nc.tensor.matmul(
                            ps_st[:, 512:1024], sst,
                            qt[0:16, c0 + 512:c0 + 1024])

                # exp(j2-1), exp(j2)
                for j in (j2, j2 + 1):
                    qe = j - 1
                    if 0 <= qe < QUADS:
                        ut = qsb.tile([128, 1024], BF16, tag="ut", bufs=7,
                                      name="ut")
                        utT[qe] = ut
                        nc.scalar.activation(ut[:], ps_stT[qe][:], AF.Exp)
                        del ps_stT[qe]

            # ---- MLP tail ----
            tbig_bf = singles.tile([128, 2 * NG], BF16)
            nc.vector.tensor_copy(tbig_bf[:], tbig[:])
            tb_r = tbig_bf[:].rearrange("p (g b) -> p b g", b=2)

            ps_z1 = pst.tile([128, BC], F32, tag="st")
            for jj in range(4):
                for i in range(4):
                    lo = 128 * jj + 32 * i
                    for b in range(2):
                        v = 8 * jj + 2 * i + b
                        nc.tensor.matmul(
                            ps_z1[:, lo:lo + 32],
                            cvar[:, 128 * v:128 * (v + 1)],
                            tb_r[:, b],
                            start=(b == 0),
                            stop=(b == 1),
                        )
            h1 = singles.tile([128, BC], BF16)
            nc.scalar.activation(h1[:], ps_z1[:], AF.Tanh, bias=b1[:])

            ps_z2 = pst.tile([128, BC], F32, tag="st")
            nc.tensor.matmul(ps_z2[:], w2[:], h1[:])
            h2 = singles.tile([128, BC], BF16)
            nc.scalar.activation(h2[:], ps_z2[:], AF.Tanh, bias=b2[:])

            ps_z3 = pg.tile([1, BC], F32, tag="gbig")
            nc.tensor.matmul(ps_z3[:], w3[:], h2[:])
            y_sb = singles.tile([1, BC], F32)
            nc.vector.tensor_add(y_sb[:], ps_z3[:], b3r[:])

            nc.sync.dma_start(
                out_t.ap().rearrange("(g jj i) o -> o jj i g", jj=4, i=4),
                y_sb[:].rearrange("o (jj i g) -> o jj i g", jj=4, i=4),
            )

    nc.compile()
    return nc


def _host_prep(inputs):
    f = lambda x: np.asarray(x, dtype=np.float32)
    s_obs = f(inputs["s_obs"])

    def aug_w(W, b):
        return np.vstack([f(W), f(b).reshape(1, -1)])  # [4, dout]

    Wq_rs = aug_w(inputs["Wq_rs"], inputs["bq_rs"])
    Wk_rs = aug_w(inputs["Wk_rs"], inputs["bk_rs"])
    Wv_rs = aug_w(inputs["Wv_rs"], inputs["bv_rs"])
    Wq_tg = aug_w(inputs["Wq_tg"], inputs["bq_tg"])
    Wk_tg = aug_w(inputs["Wk_tg"], inputs["bk_tg"])
    Wv_tg = aug_w(inputs["Wv_tg"], inputs["bv_tg"])

    scale = 1.0 / np.sqrt(16.0)
    A_rs = (Wq_rs @ Wk_rs.T * scale).astype(np.float32)   # [4, 4]
    A_tg = (Wq_tg @ Wk_tg.T * scale).astype(np.float32)

    ones = np.ones((B, N, 1), np.float32)
    s_aug = np.concatenate([s_obs, ones], axis=2)          # [B, 128, 4]

    # Y_b[item] = A_b^T s'^T : [2, B, 4, 128]
    Y = np.stack([
        np.einsum("kj,ink->ijn", A_rs, s_aug),
        np.einsum("kj,ink->ijn", A_tg, s_aug),
    ], axis=0).astype(np.float32)

    # rep128: broadcast each item's Z row (strip-local 4i+3) over its 4 rows;
    # garbage rows 16..31 of each strip read item 0's Z to stay finite.
    rep128 = np.zeros((128, 128), BF)
    for jj in range(4):
        for p in range(32):
            if p < 16:
                src = 32 * jj + 4 * (p // 4) + 3
            else:
                src = 32 * jj + 3
            rep128[src, 32 * jj + p] = 1.0

    # First MLP layer folded with Wv'/N: C_b [4, 128]
    w1 = f(inputs["W1"])                       # [64, 128]
    C_rs = (Wv_rs @ w1[0:32]) / N              # [4, 128]
    C_tg = (Wv_tg @ w1[32:64]) / N
    Cb = [C_rs, C_tg]
    # cvar [128, 4096]: variant v=(jj,i,b) at cols 128v, nonzero rows
    # 32jj+4i..32jj+4i+4 (tbig's strip layout)
    cvar = np.zeros((128, 4096), np.float32)
    for jj in range(4):
        for i in range(4):
            for b in range(2):
                v = 8 * jj + 2 * i + b
                cvar[32 * jj + 4 * i:32 * jj + 4 * (i + 1),
                     128 * v:128 * v + 128] = Cb[b]

    b1 = f(inputs["b1"]).reshape(128, 1)
    w2 = f(inputs["W2"])                       # [128, 128]
    b2 = f(inputs["b2"]).reshape(128, 1)
    w3 = f(inputs["W3"])                       # [128, 1]
    b3rep = np.full((1, BC), float(np.asarray(inputs["b3"]).reshape(-1)[0]),
                    np.float32)

    common = dict(
        rep128=rep128,
        cvar=cvar.astype(BF),
        w2=w2.astype(BF), w3=w3.astype(BF),
        b1=b1, b2=b2, b3rep=b3rep,
    )

    in_maps = []
    for c in range(N_CORES):
        lo, hi = c * BC, (c + 1) * BC
        sa = s_aug[lo:hi].reshape(QUADS, 4, N, 4)          # [Q, i, n, k]
        Yc = Y[:, lo:hi].reshape(2, QUADS, 4, 4, N)        # [b, Q, i, j, n]

        # qt [Q, (i,j)=16, (b,i',q)=1024], block-diagonal in (i, i')
        qt = np.zeros((QUADS, 4, 4, 2, 4, N), np.float32)  # q i j b i' n
        for i in range(4):
            qt[:, i, :, 0, i, :] = Yc[0, :, i]
            qt[:, i, :, 1, i, :] = Yc[1, :, i]
        qt = qt.reshape(QUADS, 16, 1024)

        # sst [Q, (i,k)=16, n=128]
        sst = sa.transpose(0, 1, 3, 2).reshape(QUADS, 16, N)

        qtss = np.concatenate([qt, sst], axis=2)           # [Q, 16, 1152]
        # qtss4 [NG, 16, 4608]: quad jj at col-block 1152*jj
        qtss4 = qtss.reshape(NG, 4, 16, 1152).transpose(0, 2, 1, 3) \
                    .reshape(NG, 16, 4608)

        # snatp4 [NG, 128, 512]: quad jj cols 128jj.., item i cols 32i..,
        # within which col 4i+k = s'_i[:, k] (maps to strip row 4i+k)
        snatp4 = np.zeros((NG, N, 4, 4, 32), np.float32)   # g kk jj i c
        sg = sa.reshape(NG, 4, 4, N, 4)                    # g jj i n k
        for i in range(4):
            snatp4[:, :, :, i, 4 * i:4 * (i + 1)] = (
                sg[:, :, i].transpose(0, 2, 1, 3))         # g kk jj k
        snatp4 = snatp4.reshape(NG, N, 512)

        m = dict(common)
        m["qtss4"] = np.ascontiguousarray(qtss4.astype(BF))
        m["snatp4"] = np.ascontiguousarray(snatp4.astype(BF))
        in_maps.append(m)
    return in_maps


def kernel(**inputs):
    if "nc" not in _cache:
        _cache["nc"] = _build()
    nc = _cache["nc"]
    in_maps = _host_prep(inputs)
    trace = os.environ.get("KERNEL_TRACE", "0") == "1"
    res = bass_utils.run_bass_kernel_spmd(
        nc, in_maps, core_ids=list(range(N_CORES)), trace=trace
    )
    _cache["last"] = res
    out = np.concatenate([r["out"] for r in res.results], axis=0)
    return out.astype(np.float32)


# revision 33
# speedup vs baseline: 1.2882x; 1.2882x over previous
"""Trainium2 Bass kernel for nn_CriticUAVob (attention-pool critic).

Math per item b (4096 total): two attention-pool branches over s_b [N=128, 3]
followed by a small MLP.  With s' = [s, 1] (N x 4) and A = Wq' Wk'^T / 4:

    S = s' A s'^T,  U = exp(S),  Z[q] = sum_kk U[q,kk]
    pooled = (1/N) * t^T Wv',  t[k] = sum_q (sum_kk U[q,kk] s'[kk,k]) / Z[q]

Per-core layout (512 items = 128 quads = 32 groups of 4 quads):
  - group inputs qtss4 [128, 1152]: quad jj of the group occupies SBUF
    partitions 32jj..32jj+16 with [qt | sst] (block-diag A^T s'^T + stacked
    s'^T); st matmuls run row-tiled (tile_position (32jj, 0)), 2 x N=512 per
    quad -> ps_st [128, 1024] = S^T with cols (b, i, q).
  - exp: one ScalarE activation [128, 1024] per quad (identity layout) ->
    ut bf16.  This is the kernel's roofline: ~1.05us x 128 quads.
  - G matmuls: 4 per quad, lhsT = zero-padded s' variants [128, 32], output
    col-strip 32jj of a shared gbig [128, 256] PSUM tile (cols (b, q)),
    accumulating all 4 items; the s' ones-column makes row (i,3) = Z.
  - per-group chain on full-width [128, 256] tiles: bf16 cast (DVE), rep128
    matmul broadcasts each item's Z row over its 4 rows (PE), fast reciprocal
    (DVE), then two fused tensor_tensor_reduce (DVE) produce t columns of
    tbig [128, 64] directly.
  - MLP: first layer folds Wv'/N and W1 into C [4,128]; 32 row-tiled matmuls
    with zero-padded C variants read tbig in place; tanh/tanh/linear finish,
    and the item permutation (jj, i, g) is undone for free in the output DMA.
"""
import os
import sys
import numpy as np
import ml_dtypes

sys.path.insert(0, "/opt/trn_rl_repo")

import concourse.bass as bass
import concourse.tile as tile
from concourse import bacc, mybir
from concourse import bass_utils

N_CORES = 8
B = 4096
N = 128
BC = B // N_CORES          # 512 items per core
QUADS = BC // 4            # 128 quads of 4 items
NG = QUADS // 4            # 32 groups of 4 quads
F32 = mybir.dt.float32
BF16 = mybir.dt.bfloat16
AF = mybir.ActivationFunctionType
ALU = mybir.AluOpType
BF = ml_dtypes.bfloat16

_cache = {}


def _build():
    nc = bacc.Bacc(
        "TRN2",
        target_bir_lowering=False,
        debug=False,
        enable_asserts=False,
        num_devices=N_CORES,
    )
    qtss4_t = nc.dram_tensor("qtss4", [NG, 16, 4608], BF16, kind="ExternalInput")
    qtss4z_t = nc.dram_tensor("qtss4z", [3, 128, 4608], BF16,
                              kind="ExternalInput")
    snatp4_t = nc.dram_tensor("snatp4", [NG, 128, 2048], BF16,
                              kind="ExternalInput")
    rep_t = nc.dram_tensor("rep128", [128, 128], BF16, kind="ExternalInput")
    cvar_t = nc.dram_tensor("cvar", [128, 4096], BF16, kind="ExternalInput")
    w2_t = nc.dram_tensor("w2", [128, 128], BF16, kind="ExternalInput")
    w3_t = nc.dram_tensor("w3", [128, 1], BF16, kind="ExternalInput")
    b1_t = nc.dram_tensor("b1", [128, 1], F32, kind="ExternalInput")
    b2_t = nc.dram_tensor("b2", [128, 1], F32, kind="ExternalInput")
    b3_t = nc.dram_tensor("b3rep", [1, BC], F32, kind="ExternalInput")
    out_t = nc.dram_tensor("out", [BC, 1], F32, kind="ExternalOutput")

    qtss4_ap = qtss4_t.ap()
    qtss4z_ap = qtss4z_t.ap()
    snatp4_ap = snatp4_t.ap()

    with tile.TileContext(nc) as tc:
        with (
            tc.tile_pool(name="singles", bufs=1) as singles,
            tc.tile_pool(name="qsb", bufs=2) as qsb,
            tc.tile_pool(name="pst", bufs=3, space="PSUM") as pst,
            tc.tile_pool(name="pg", bufs=2, space="PSUM") as pg,
        ):
            # singles DMAs are deferred into the loop on the gpsimd queue so
            # neither the scalar queue (exp stream) nor the first qtss/snatp
            # loads are delayed
            rep128 = singles.tile([128, 128], BF16)
            cvar = singles.tile([128, 4096], BF16)
            w2 = singles.tile([128, 128], BF16)
            w3 = singles.tile([128, 1], BF16)
            b1 = singles.tile([128, 1], F32)
            b2 = singles.tile([128, 1], F32)
            b3r = singles.tile([1, BC], F32)
            single_dmas = [
                (rep128, rep_t), (cvar, cvar_t), (w2, w2_t), (w3, w3_t),
                (b1, b1_t), (b2, b2_t), (b3r, b3_t),
            ]
            # t accumulator: rows (jj, i, k) in 32-strips, cols (g, b)
            tbig = singles.tile([128, 2 * NG], F32)

            qtssT, snatpT, ps_stT, utT, gbigT, gcastT, rbigT = (
                {}, {}, {}, {}, {}, {}, {},
            )

            def issue_dma(g):
                # [128, .] tiles: rows 16-127 of the qtss slots are zeroed by
                # the first three full-size DMAs and persist across slot reuse
                # (later DMAs only write rows 0:16) -> st matmuls can run as
                # full 128-contraction (no row tiling, no PE mode switches).
                qtssT[g] = qsb.tile([128, 4608], BF16, tag="qtss", bufs=3,
                                    name="qtss")
                if g < 3:
                    nc.sync.dma_start(qtssT[g][:], qtss4z_ap[g])
                else:
                    nc.sync.dma_start(qtssT[g][0:16, :], qtss4_ap[g])
                snatpT[g] = qsb.tile([128, 2048], BF16, tag="snatp", bufs=4,
                                     name="snatp")
                nc.gpsimd.dma_start(snatpT[g][:], snatp4_ap[g])

            # Software pipeline over iteration PAIRS: per pair (j, j+1) the PE
            # runs [rep (if due)] [8 G matmuls] [4 st matmuls] — quad q's
            # G-item-i matmul lands at iteration q+2+i, so consecutive G
            # matmuls come from different quads = distinct PSUM col-strips and
            # stream concurrently in the PE array (array packing).  Pairing
            # halves the row/col-tiling mode-switch drains.
            for j2 in range(0, QUADS + 12, 2):
                if j2 < QUADS and j2 % 4 == 0:
                    g = j2 // 4
                    if g == 0:
                        issue_dma(0)
                    if g + 1 < NG:
                        issue_dma(g + 1)
                if j2 == 2:
                    for tl, dt_ in single_dmas:
                        nc.gpsimd.dma_start(tl[:], dt_.ap())

                # chain part 2 first (correct WAR order vs gbig slot reuse):
                # rep (PE) reads gcast from the previous pair, then recip +
                # mul/reduce (DVE).  zrep shares gbig's PSUM bank.
                for j in (j2, j2 + 1):
                    if j >= 10 and (j - 10) % 4 == 0 and (j - 10) // 4 < NG:
                        gr = (j - 10) // 4
                        nc.tensor.matmul(gbigT[gr][:, 256:512], rep128[:],
                                         gcastT[gr][:])
                        rbig = qsb.tile([128, 256], F32, tag="rbig",
                                        name="rbig")
                        rbigT[gr] = rbig
                        nc.vector.reciprocal_approx_fast(
                            rbig[:], gbigT[gr][:, 256:512])
                        pgm = qsb.tile([128, 256], F32, tag="scr", name="pgm")
                        nc.vector.tensor_mul(pgm[:], gbigT[gr][:, 0:256],
                                             rbig[:])
                        pg3 = pgm[:].rearrange("p (b q) -> p b q", b=2)
                        nc.vector.tensor_reduce(
                            tbig[:, 2 * gr:2 * (gr + 1)], pg3,
                            axis=mybir.AxisListType.X, op=ALU.add,
                        )
                        del gbigT[gr], gcastT[gr], rbigT[gr]

                # G: 8 matmuls (item i of quad j-3-i for both iterations)
                for j in (j2, j2 + 1):
                    for i in range(4):
                        qg = j - 3 - i
                        if not (0 <= qg < QUADS):
                            continue
                        gg, jj = qg // 4, qg % 4
                        if jj == 0 and i == 0:
                            gbigT[gg] = pg.tile([128, 512], F32, tag="gbig",
                                                name="gbig")
                        gbig = gbigT[gg]
                        ut_r = utT[qg][:].rearrange("p (b i q) -> p i b q",
                                                    b=2, i=4)
                        sn = snatpT[gg]
                        v = 4 * jj + i
                        nc.tensor.matmul(
                            gbig[:, 0:256],
                            sn[:, 128 * v:128 * (v + 1)],
                            ut_r[:, i],
                            start=(jj == 0 and i == 0),
                            stop=(jj == 3 and i == 3),
                        )
                        if i == 3:
                            del utT[qg]
                            if jj == 3:
                                # chain part 1: bf16 cast for rep's rhs
                                gcast = qsb.tile([128, 256], BF16,
                                                 tag="gcast", name="gcast")
                                gcastT[gg] = gcast
                                nc.vector.tensor_copy(gcast[:],
                                                      gbig[:, 0:256])

                # st(j2), st(j2+1): two matmuls each
                for j in (j2, j2 + 1):
                    if j < QUADS:
                        g, jj = j // 4, j % 4
                        qt = qtssT[g]
                        c0 = 1152 * jj
                        sst = qt[:, c0 + 1024:c0 + 1152]
                        ps_st = pst.tile([128, 1024], F32, tag="st",
                                         name="ps_st")
                        ps_stT[j] = ps_st
                        nc.tensor.matmul(
                            ps_st[:, 0:512], sst, qt[:, c0:c0 + 512])
                        nc.tensor.matmul(
                            ps_st[:, 512:1024], sst,
                            qt[:, c0 + 512:c0 + 1024])

                # exp(j2-1), exp(j2)
                for j in (j2, j2 + 1):
                    qe = j - 1
                    if 0 <= qe < QUADS:
                        ut = qsb.tile([128, 1024], BF16, tag="ut", bufs=7,
                                      name="ut")
                        utT[qe] = ut
                        nc.scalar.activation(ut[:], ps_stT[qe][:], AF.Exp)
                        del ps_stT[qe]

            # ---- MLP tail ----
            tbig_bf = singles.tile([128, 2 * NG], BF16)
            nc.vector.tensor_copy(tbig_bf[:], tbig[:])
            tb_r = tbig_bf[:].rearrange("p (g b) -> p b g", b=2)

            ps_z1 = pst.tile([128, BC], F32, tag="st")
            for jj in range(4):
                for i in range(4):
                    lo = 128 * jj + 32 * i
                    for b in range(2):
                        v = 8 * jj + 2 * i + b
                        nc.tensor.matmul(
                            ps_z1[:, lo:lo + 32],
                            cvar[:, 128 * v:128 * (v + 1)],
                            tb_r[:, b],
                            start=(b == 0),
                            stop=(b == 1),
                        )
            h1 = singles.tile([128, BC], BF16)
            nc.scalar.activation(h1[:], ps_z1[:], AF.Tanh, bias=b1[:])

            ps_z2 = pst.tile([128, BC], F32, tag="st")
            nc.tensor.matmul(ps_z2[:], w2[:], h1[:])
            h2 = singles.tile([128, BC], BF16)
            nc.scalar.activation(h2[:], ps_z2[:], AF.Tanh, bias=b2[:])

            ps_z3 = pg.tile([1, BC], F32, tag="gbig")
            nc.tensor.matmul(ps_z3[:], w3[:], h2[:])
            y_sb = singles.tile([1, BC], F32)
            nc.vector.tensor_add(y_sb[:], ps_z3[:], b3r[:])

            nc.sync.dma_start(
                out_t.ap().rearrange("(g jj i) o -> o jj i g", jj=4, i=4),
                y_sb[:].rearrange("o (jj i g) -> o jj i g", jj=4, i=4),
            )

    nc.compile()
    return nc


def _host_prep(inputs):
    f = lambda x: np.asarray(x, dtype=np.float32)
    s_obs = f(inputs["s_obs"])

    def aug_w(W, b):
        return np.vstack([f(W), f(b).reshape(1, -1)])  # [4, dout]

    Wq_rs = aug_w(inputs["Wq_rs"], inputs["bq_rs"])
    Wk_rs = aug_w(inputs["Wk_rs"], inputs["bk_rs"])
    Wv_rs = aug_w(inputs["Wv_rs"], inputs["bv_rs"])
    Wq_tg = aug_w(inputs["Wq_tg"], inputs["bq_tg"])
    Wk_tg = aug_w(inputs["Wk_tg"], inputs["bk_tg"])
    Wv_tg = aug_w(inputs["Wv_tg"], inputs["bv_tg"])

    scale = 1.0 / np.sqrt(16.0)
    A_rs = (Wq_rs @ Wk_rs.T * scale).astype(np.float32)   # [4, 4]
    A_tg = (Wq_tg @ Wk_tg.T * scale).astype(np.float32)

    ones = np.ones((B, N, 1), np.float32)
    s_aug = np.concatenate([s_obs, ones], axis=2)          # [B, 128, 4]

    # Y_b[item] = A_b^T s'^T : [2, B, 4, 128]
    Y = np.stack([
        np.einsum("kj,ink->ijn", A_rs, s_aug),
        np.einsum("kj,ink->ijn", A_tg, s_aug),
    ], axis=0).astype(np.float32)

    # rep128: broadcast each item's Z row (strip-local 4i+3) over its 4 rows;
    # garbage rows 16..31 of each strip read item 0's Z to stay finite.
    rep128 = np.zeros((128, 128), BF)
    for jj in range(4):
        for p in range(32):
            if p < 16:
                src = 32 * jj + 4 * (p // 4) + 3
            else:
                src = 32 * jj + 3
            rep128[src, 32 * jj + p] = 1.0

    # First MLP layer folded with Wv'/N: C_b [4, 128]
    w1 = f(inputs["W1"])                       # [64, 128]
    C_rs = (Wv_rs @ w1[0:32]) / N              # [4, 128]
    C_tg = (Wv_tg @ w1[32:64]) / N
    Cb = [C_rs, C_tg]
    # cvar [128, 4096]: variant v=(jj,i,b) at cols 128v, nonzero rows
    # 32jj+4i..32jj+4i+4 (tbig's strip layout)
    cvar = np.zeros((128, 4096), np.float32)
    for jj in range(4):
        for i in range(4):
            for b in range(2):
                v = 8 * jj + 2 * i + b
                cvar[32 * jj + 4 * i:32 * jj + 4 * (i + 1),
                     128 * v:128 * v + 128] = Cb[b]

    b1 = f(inputs["b1"]).reshape(128, 1)
    w2 = f(inputs["W2"])                       # [128, 128]
    b2 = f(inputs["b2"]).reshape(128, 1)
    w3 = f(inputs["W3"])                       # [128, 1]
    b3rep = np.full((1, BC), float(np.asarray(inputs["b3"]).reshape(-1)[0]),
                    np.float32)

    common = dict(
        rep128=rep128,
        cvar=cvar.astype(BF),
        w2=w2.astype(BF), w3=w3.astype(BF),
        b1=b1, b2=b2, b3rep=b3rep,
    )

    in_maps = []
    for c in range(N_CORES):
        lo, hi = c * BC, (c + 1) * BC
        sa = s_aug[lo:hi].reshape(QUADS, 4, N, 4)          # [Q, i, n, k]
        Yc = Y[:, lo:hi].reshape(2, QUADS, 4, 4, N)        # [b, Q, i, j, n]

        # qt [Q, (i,j)=16, (b,i',q)=1024], block-diagonal in (i, i')
        qt = np.zeros((QUADS, 4, 4, 2, 4, N), np.float32)  # q i j b i' n
        for i in range(4):
            qt[:, i, :, 0, i, :] = Yc[0, :, i]
            qt[:, i, :, 1, i, :] = Yc[1, :, i]
        qt = qt.reshape(QUADS, 16, 1024)

        # sst [Q, (i,k)=16, n=128]
        sst = sa.transpose(0, 1, 3, 2).reshape(QUADS, 16, N)

        qtss = np.concatenate([qt, sst], axis=2)           # [Q, 16, 1152]
        # qtss4 [NG, 16, 4608]: quad jj at col-block 1152*jj
        qtss4 = qtss.reshape(NG, 4, 16, 1152).transpose(0, 2, 1, 3) \
                    .reshape(NG, 16, 4608)

        # snatp4 [NG, 128, 2048]: variant v=(jj,i) is a full-width [128,128]
        # zero-padded stationary with s'_i at cols 32jj+4i..+4 (gbig rows)
        snatp4 = np.zeros((NG, N, 16, 128), np.float32)    # g kk v c
        sg = sa.reshape(NG, 4, 4, N, 4)                    # g jj i n k
        for jj in range(4):
            for i in range(4):
                v = 4 * jj + i
                c0 = 32 * jj + 4 * i
                snatp4[:, :, v, c0:c0 + 4] = sg[:, jj, i]  # g kk k
        snatp4 = snatp4.reshape(NG, N, 2048)

        m = dict(common)
        m["qtss4"] = np.ascontiguousarray(qtss4.astype(BF))
        qz = np.zeros((3, 128, 4608), np.float32)
        qz[:, 0:16, :] = qtss4[0:3]
        m["qtss4z"] = np.ascontiguousarray(qz.astype(BF))
        m["snatp4"] = np.ascontiguousarray(snatp4.astype(BF))
        in_maps.append(m)
    return in_maps


def kernel(**inputs):
    if "nc" not in _cache:
        _cache["nc"] = _build()
    nc = _cache["nc"]
    in_maps = _host_prep(inputs)
    trace = os.environ.get("KERNEL_TRACE", "0") == "1"
    res = bass_utils.run_bass_kernel_spmd(
        nc, in_maps, core_ids=list(range(N_CORES)), trace=trace
    )
    _cache["last"] = res
    out = np.concatenate([r["out"] for r in res.results], axis=0)
    return out.astype(np.float32)
